# revision 4
# baseline (speedup 1.0000x reference)
"""BiAttention kernel for 8 TRN2 NeuronCores (batch data-parallel).

B=16, LD=4096, LM=1024, H=128.  Each core handles 2 batch items.

Math (per batch):
  id[i]  = x[i,:]@w_i1          md[m] = mem[m,:]@w_m1
  att    = id[:,None] + md[None,:] + x@mem.T/sqrt(H)
  p      = exp(md + cross_scaled)            (id cancels in softmax over m)
  out1   = (p @ mem2raw)/sums + b_m2         (softmax weights sum to 1)
  pmax   = max_m p ;  wtu = exp(id)*pmax ;  wt = wtu/Z
  out2   = sum_i wt[i]*inp2raw[i,:] + b_i2
  out    = inp2b@Wo1.T + out1@(Wo2 + out2*Wo4).T + (inp2b*out1)@Wo3.T + b_out

Layouts on chip (all bf16 matmul operands, f32 PSUM):
  xT   [d=128, i]   memT [d=128, m]    p [m-part, i-free] per span
"""

import numpy as np
import ml_dtypes

import concourse.bass as bass
import concourse.tile as tile
from concourse import bacc, mybir

F32 = mybir.dt.float32
BF16 = mybir.dt.bfloat16
AF = mybir.ActivationFunctionType
ALU = mybir.AluOpType

B_CORE = 2
LD, LM, H = 4096, 1024, 128
SPAN = 512
NSPAN = LD // SPAN          # 8
NCH = LM // 128             # 8
NBLK = SPAN // 128          # 4
NB32 = LD // 128            # 32 blocks per batch
SCALE = float(1.0 / np.sqrt(H))

_CACHE = {}


def _build():
    nc = bacc.Bacc(None, target_bir_lowering=False)

    x = nc.dram_tensor("x", [B_CORE, LD, H], F32, kind="ExternalInput")
    mem = nc.dram_tensor("mem", [B_CORE, LM, H], F32, kind="ExternalInput")
    wi1 = nc.dram_tensor("wi1", [H, 1], BF16, kind="ExternalInput")
    wm1 = nc.dram_tensor("wm1", [H, 1], BF16, kind="ExternalInput")
    wi2t = nc.dram_tensor("wi2t", [H, H], BF16, kind="ExternalInput")
    wm2t = nc.dram_tensor("wm2t", [H, H], BF16, kind="ExternalInput")
    wot = nc.dram_tensor("wot", [4 * H, H], BF16, kind="ExternalInput")
    bi2 = nc.dram_tensor("bi2", [H, 1], F32, kind="ExternalInput")
    bi2r = nc.dram_tensor("bi2r", [1, H], F32, kind="ExternalInput")
    bm2 = nc.dram_tensor("bm2", [H, 1], F32, kind="ExternalInput")
    boutr = nc.dram_tensor("boutr", [1, H], BF16, kind="ExternalInput")
    ones1 = nc.dram_tensor("ones1", [1, H], BF16, kind="ExternalInput")
    ones = nc.dram_tensor("ones", [H, H], BF16, kind="ExternalInput")
    ident = nc.dram_tensor("ident", [H, H], BF16, kind="ExternalInput")
    out = nc.dram_tensor("out", [B_CORE, LD, H], F32, kind="ExternalOutput")

    with tile.TileContext(nc) as tc:
        with (
            tc.tile_pool(name="const", bufs=1) as cpool,
            tc.tile_pool(name="keep", bufs=1) as kpool,
            tc.tile_pool(name="work", bufs=3) as wpool,
            tc.tile_pool(name="pspan", bufs=2) as ppool,
            tc.tile_pool(name="psum", bufs=1, space="PSUM") as qpool,
            tc.tile_pool(name="psatt", bufs=2, space="PSUM") as qatt,
            tc.tile_pool(name="pstp", bufs=2, space="PSUM") as qtp,
        ):
            # ---- constants ----
            t_wi1 = cpool.tile([H, 1], BF16, tag="c0")
            t_wm1 = cpool.tile([H, 1], BF16, tag="c1")
            t_wi2t = cpool.tile([H, H], BF16, tag="c2")
            t_wm2t = cpool.tile([H, H], BF16, tag="c3")
            t_wo = [cpool.tile([H, H], BF16, tag=f"c4{j}", name=f"t_wo{j}")
                    for j in range(4)]
            t_bi2 = cpool.tile([H, 1], F32, tag="c5")
            t_bi2r = cpool.tile([1, H], F32, tag="c6")
            t_bm2 = cpool.tile([H, 1], F32, tag="c7")
            t_boutr = cpool.tile([1, H], BF16, tag="c8")
            t_ones1 = cpool.tile([1, H], BF16, tag="c9")
            t_ones = cpool.tile([H, H], BF16, tag="c10")
            t_id = cpool.tile([H, H], BF16, tag="c11")
            nc.sync.dma_start(t_wi1[:], wi1[:, :])
            nc.sync.dma_start(t_wm1[:], wm1[:, :])
            nc.sync.dma_start(t_wi2t[:], wi2t[:, :])
            nc.sync.dma_start(t_wm2t[:], wm2t[:, :])
            for j in range(4):
                nc.sync.dma_start(t_wo[j][:], wot[j * H:(j + 1) * H, :])
            nc.sync.dma_start(t_bi2[:], bi2[:, :])
            nc.sync.dma_start(t_bi2r[:], bi2r[:, :])
            nc.sync.dma_start(t_bm2[:], bm2[:, :])
            nc.sync.dma_start(t_boutr[:], boutr[:, :])
            nc.sync.dma_start(t_ones1[:], ones1[:, :])
            nc.sync.dma_start(t_ones[:], ones[:, :])
            nc.sync.dma_start(t_id[:], ident[:, :])

            for b in range(B_CORE):
                # ---- per-batch keepers ----
                memT = kpool.tile([H, LM], BF16, tag="memT")
                mem2 = kpool.tile([H, NCH * H], BF16, tag="mem2")
                mdsb = kpool.tile([H, NCH], F32, tag="mdsb")
                xTf = kpool.tile([H, LD], BF16, tag="xTf")
                inp2bT = kpool.tile([H, LD], BF16, tag="inp2bT")
                out1nT = kpool.tile([H, LD], BF16, tag="out1nT")
                cat3T = kpool.tile([H, LD], BF16, tag="cat3T")
                eid = kpool.tile([H, NB32], F32, tag="eid")
                pmaxv = kpool.tile([H, NB32], F32, tag="pmaxv")

                # ---- mem prep: memT, md, mem2 ----
                for c in range(NCH):
                    mn = wpool.tile([128, H], BF16, tag="mnat")
                    nc.gpsimd.dma_start(mn[:], mem[b, c * 128:(c + 1) * 128, :])
                    tp = qtp.tile([128, 128], BF16, tag="tps")
                    nc.tensor.transpose(tp[:], mn[:], t_id[:])
                    nc.vector.tensor_copy(memT[:, c * 128:(c + 1) * 128], tp[:])
                mdps = qpool.tile([128, NCH], F32, tag="sfin")
                for c in range(NCH):
                    nc.tensor.matmul(mdps[:, c:c + 1],
                                     memT[:, c * 128:(c + 1) * 128],
                                     t_wm1[:], start=True, stop=True)
                nc.vector.tensor_copy(mdsb[:], mdps[:])
                for c in range(NCH):
                    m2ps = qtp.tile([128, 128], F32, tag="tps")
                    nc.tensor.matmul(m2ps[:], memT[:, c * 128:(c + 1) * 128],
                                     t_wm2t[:], start=True, stop=True)
                    nc.vector.tensor_copy(mem2[:, c * 128:(c + 1) * 128], m2ps[:])

                # ---- pass 1 over spans ----
                for s in range(NSPAN):
                    i0 = s * SPAN
                    for t in range(NBLK):
                        xn = wpool.tile([128, H], BF16, tag="xnat")
                        r0 = i0 + t * 128
                        nc.gpsimd.dma_start(xn[:], x[b, r0:r0 + 128, :])
                        tp = qtp.tile([128, 128], BF16, tag="tps")
                        nc.tensor.transpose(tp[:], xn[:], t_id[:])
                        nc.vector.tensor_copy(xTf[:, r0:r0 + 128], tp[:])
                    xT = xTf[:, i0:i0 + SPAN]

                    # id -> eid   [128 i-part, 1] per 128-block
                    idp = qpool.tile([128, NBLK], F32, tag="sfin")
                    for t in range(NBLK):
                        nc.tensor.matmul(idp[:, t:t + 1],
                                         xTf[:, i0 + t * 128:i0 + (t + 1) * 128],
                                         t_wi1[:], start=True, stop=True)
                    nc.scalar.activation(eid[:, s * NBLK:(s + 1) * NBLK], idp[:],
                                         AF.Exp)

                    # inp2T + bias
                    i2ps = qpool.tile([128, SPAN], F32, tag="i2p")
                    nc.tensor.matmul(i2ps[:], t_wi2t[:], xT, start=True, stop=True)
                    nc.vector.tensor_scalar_add(inp2bT[:, i0:i0 + SPAN], i2ps[:],
                                                t_bi2[:])

                    # cross, exp, out1u, sums
                    p_sp = ppool.tile([128, NCH * SPAN], BF16, tag="p")
                    o1ps = qpool.tile([128, SPAN], F32, tag="out1")
                    smps = qpool.tile([128, SPAN], F32, tag="sums")
                    for c in range(NCH):
                        attps = qatt.tile([128, SPAN], F32, tag="att")
                        nc.tensor.matmul(attps[:], memT[:, c * 128:(c + 1) * 128],
                                         xT, start=True, stop=True)
                        nc.scalar.activation(p_sp[:, c * SPAN:(c + 1) * SPAN],
                                             attps[:], AF.Exp,
                                             bias=mdsb[:, c:c + 1], scale=SCALE)
                        nc.tensor.matmul(o1ps[:], mem2[:, c * 128:(c + 1) * 128],
                                         p_sp[:, c * SPAN:(c + 1) * SPAN],
                                         start=(c == 0), stop=(c == NCH - 1))
                        nc.tensor.matmul(smps[:], t_ones[:],
                                         p_sp[:, c * SPAN:(c + 1) * SPAN],
                                         start=(c == 0), stop=(c == NCH - 1))

                    # pmax tree over the 8 chunks (DVE), then transpose+reduce
                    tr1 = wpool.tile([128, 4 * SPAN], BF16, tag="tr1")
                    nc.vector.tensor_max(tr1[:], p_sp[:, 0:4 * SPAN],
                                         p_sp[:, 4 * SPAN:8 * SPAN])
                    tr2 = wpool.tile([128, 2 * SPAN], BF16, tag="tr2")
                    nc.vector.tensor_max(tr2[:], tr1[:, 0:2 * SPAN],
                                         tr1[:, 2 * SPAN:4 * SPAN])
                    tr3 = wpool.tile([128, SPAN], BF16, tag="tr3")
                    nc.vector.tensor_max(tr3[:], tr2[:, 0:SPAN],
                                         tr2[:, SPAN:2 * SPAN])
                    sfin = qpool.tile([128, SPAN], BF16, tag="sfin")
                    for t in range(NBLK):
                        nc.tensor.transpose(sfin[:, t * 128:(t + 1) * 128],
                                            tr3[:, t * 128:(t + 1) * 128], t_id[:])
                    nc.vector.tensor_reduce(
                        pmaxv[:, s * NBLK:(s + 1) * NBLK],
                        sfin[:].rearrange("p (t m) -> p t m", t=NBLK),
                        axis=mybir.AxisListType.X, op=ALU.max)

                    # normalize out1, fold b_m2; cat3
                    invs = wpool.tile([128, SPAN], F32, tag="invs")
                    nc.vector.reciprocal(invs[:], smps[:])
                    o1tmp = wpool.tile([128, SPAN], BF16, tag="o1tmp")
                    nc.vector.tensor_mul(o1tmp[:], o1ps[:], invs[:])
                    nc.vector.tensor_scalar_add(out1nT[:, i0:i0 + SPAN], o1tmp[:],
                                                t_bm2[:])
                    nc.vector.tensor_mul(cat3T[:, i0:i0 + SPAN],
                                         inp2bT[:, i0:i0 + SPAN],
                                         out1nT[:, i0:i0 + SPAN])

                # ---- out2 ----
                wtuf = wpool.tile([128, NB32], F32, tag="wtuf")
                nc.vector.tensor_mul(wtuf[:], eid[:], pmaxv[:])
                wtub = wpool.tile([128, NB32], BF16, tag="wtub")
                nc.vector.tensor_copy(wtub[:], wtuf[:])
                zred = wpool.tile([128, 1], F32, tag="zred")
                nc.vector.tensor_reduce(zred[:], wtuf[:],
                                        axis=mybir.AxisListType.X, op=ALU.add)
                zredb = wpool.tile([128, 1], BF16, tag="zredb")
                nc.vector.tensor_copy(zredb[:], zred[:])
                zps = qpool.tile([1, 1], F32, tag="sfin")
                nc.tensor.matmul(zps[:], zredb[:], t_ones[:, 0:1],
                                 start=True, stop=True)
                zr = wpool.tile([1, 1], F32, tag="zr")
                nc.vector.reciprocal(zr[:], zps[:])

                numps = qpool.tile([1, H], F32, tag="sums")
                for blk in range(NB32):
                    i2nps = qtp.tile([128, 128], F32, tag="tps")
                    nc.tensor.matmul(i2nps[:], xTf[:, blk * 128:(blk + 1) * 128],
                                     t_wi2t[:], start=True, stop=True)
                    i2nsb = wpool.tile([128, 128], BF16, tag="i2n")
                    nc.vector.tensor_copy(i2nsb[:], i2nps[:])
                    nc.tensor.matmul(numps[:], wtub[:, blk:blk + 1], i2nsb[:],
                                     start=(blk == 0), stop=(blk == NB32 - 1))

                o2a = wpool.tile([1, H], F32, tag="o2a")
                nc.vector.tensor_scalar_mul(o2a[:], numps[:], zr[:])
                o2b = wpool.tile([1, H], F32, tag="o2b")
                nc.vector.tensor_add(o2b[:], o2a[:], t_bi2r[:])
                o2bf = wpool.tile([1, H], BF16, tag="o2bf")
                nc.vector.tensor_copy(o2bf[:], o2b[:])
                otps = qpool.tile([H, 1], F32, tag="sfin")
                nc.tensor.matmul(otps[:], o2bf[:], t_ones1[:, 0:1],
                                 start=True, stop=True)
                otsb = wpool.tile([H, 1], F32, tag="otsb")
                nc.vector.tensor_copy(otsb[:], otps[:])
                w4s = wpool.tile([H, H], BF16, tag="w4s")
                nc.vector.tensor_scalar_mul(w4s[:], t_wo[3][:], otsb[:])
                rhs24 = wpool.tile([H, H], BF16, tag="rhs24")
                nc.vector.tensor_add(rhs24[:], w4s[:], t_wo[1][:])

                # ---- pass 2: final output ----
                for blk in range(NB32):
                    j0 = blk * 128
                    ops_ = qatt.tile([128, 128], F32, tag="att")
                    nc.tensor.matmul(ops_[:], inp2bT[:, j0:j0 + 128], t_wo[0][:],
                                     start=True, stop=False)
                    nc.tensor.matmul(ops_[:], out1nT[:, j0:j0 + 128], rhs24[:],
                                     start=False, stop=False)
                    nc.tensor.matmul(ops_[:], cat3T[:, j0:j0 + 128], t_wo[2][:],
                                     start=False, stop=False)
                    nc.tensor.matmul(ops_[:], t_ones1[:], t_boutr[:],
                                     start=False, stop=True)
                    osb = wpool.tile([128, 128], F32, tag="osb")
                    if blk % 2 == 0:
                        nc.vector.tensor_copy(osb[:], ops_[:])
                    else:
                        nc.scalar.copy(osb[:], ops_[:])
                    nc.sync.dma_start(out[b, j0:j0 + 128, :], osb[:])
    nc.finalize()
    return nc


def _prep_params(w_i1, w_m1, W_i2, b_i2, W_m2, b_m2, W_out, b_out):
    bf = ml_dtypes.bfloat16
    f32 = np.float32
    return {
        "wi1": np.ascontiguousarray(w_i1.reshape(H, 1)).astype(bf),
        "wm1": np.ascontiguousarray(w_m1.reshape(H, 1)).astype(bf),
        "wi2t": np.ascontiguousarray(np.asarray(W_i2).T).astype(bf),
        "wm2t": np.ascontiguousarray(np.asarray(W_m2).T).astype(bf),
        "wot": np.ascontiguousarray(np.asarray(W_out).T).astype(bf),
        "bi2": np.ascontiguousarray(b_i2.reshape(H, 1)).astype(f32),
        "bi2r": np.ascontiguousarray(b_i2.reshape(1, H)).astype(f32),
        "bm2": np.ascontiguousarray(b_m2.reshape(H, 1)).astype(f32),
        "boutr": np.ascontiguousarray(b_out.reshape(1, H)).astype(bf),
        "ones1": np.ones((1, H), dtype=bf),
        "ones": np.ones((H, H), dtype=bf),
        "ident": np.eye(H, dtype=np.float32).astype(bf),
    }


def kernel(x, mem, w_i1, w_m1, W_i2, b_i2, W_m2, b_m2, W_out, b_out):
    from concourse.bass_utils import run_bass_kernel_spmd

    if "nc" not in _CACHE:
        _CACHE["nc"] = _build()
    nc = _CACHE["nc"]

    x = np.ascontiguousarray(np.asarray(x), dtype=np.float32)
    mem = np.ascontiguousarray(np.asarray(mem), dtype=np.float32)
    params = _prep_params(np.asarray(w_i1, np.float32), np.asarray(w_m1, np.float32),
                          np.asarray(W_i2, np.float32), np.asarray(b_i2, np.float32),
                          np.asarray(W_m2, np.float32), np.asarray(b_m2, np.float32),
                          np.asarray(W_out, np.float32), np.asarray(b_out, np.float32))
    in_maps = []
    for cid in range(8):
        m = {"x": np.ascontiguousarray(x[cid * B_CORE:(cid + 1) * B_CORE]),
             "mem": np.ascontiguousarray(mem[cid * B_CORE:(cid + 1) * B_CORE])}
        m.update(params)
        in_maps.append(m)
    res = run_bass_kernel_spmd(nc, in_maps, core_ids=list(range(8)),
                               **_CACHE.get("extra", {}))
    _CACHE["last_result"] = res
    outs = [r["out"] for r in res.results]
    return np.concatenate(outs, axis=0).astype(np.float32)
